# revision 11
# baseline (speedup 1.0000x reference)
"""Clockwork-RNN Trainium2 kernel (8-core data-parallel over batch).

Per-core layout (B_local = 32 batch columns per core):
  - Host pre-transposes X to d-major ("XT" [DIN, T*BL], bf16) and weights to
    lhsT layouts (WiT/WhT/WoT), so no on-chip transposes are needed.
  - Recurrence runs as a serial chain per timestep: PE matmuls accumulate the
    pre-activation for the *active prefix* of hidden blocks (clockwork
    schedule => active rows are always a prefix), ACT applies tanh and
    overwrites the prefix of the state (binary {0,1} gating == prefix
    overwrite, so no gating arithmetic is needed).
  - tanh writes straight into the per-chunk history buffer hc[*, ktile, s, b];
    the recurrence reads its operands back out of hc at statically-known
    columns (block0 from s-1, block1 from the last even step, slow blocks
    from the last quad step), so there are no per-step state copies at all.
  - Matmuls with no dependency on the previous step's tanh are emitted
    *before* that tanh's chain matmul, so the in-order PE sequencer
    dispatches them during the ACT latency (software pipelining).
  - Slow blocks inactive at a quad step are carried forward by off-chain DVE
    copies; non-quad history columns are bulk-filled at chunk end with
    strided copies (only the output projection reads those).
  - Output projection is computed in flipped orientation (Y [DOUT, T*BL]) so
    Wo stays the stationary operand and the per-partition bias trick works;
    the host transposes Y back. Its tanh is split into halves that hide in
    ACT's idle gaps between chain tanhs.
  - Everything except psum/tanh/Y runs in bf16 (rel err ~3.5e-3 end-to-end).
"""

import numpy as np
import ml_dtypes

T, B, DIN = 1024, 256, 256
NSTATES = 64
NCLOCKS = 8
HID = NCLOCKS * NSTATES  # 512
DOUT = 256
N_CORES = 8
BL = B // N_CORES  # 32
TC = 64            # timesteps per chunk
NCH = T // TC

F32 = None  # set in _build (mybir import deferred)
BF16 = None

_CACHE = {}


def _active_blocks(t: int) -> int:
    if t == 0:
        return NCLOCKS
    return min((t & -t).bit_length() - 1, NCLOCKS - 1) + 1


def _build_program(nrep=1):
    """Build the (SPMD) bass program once; returns (nc, meta)."""
    import concourse.bacc as bacc
    import concourse.tile as tile
    import concourse.mybir as mybir
    from concourse.bass_utils import run_bass_kernel_spmd  # noqa: F401

    f32 = mybir.dt.float32
    bf16 = mybir.dt.bfloat16
    TANH = mybir.ActivationFunctionType.Tanh

    nc = bacc.Bacc("TRN2", target_bir_lowering=False, debug=False)

    xt_d = nc.dram_tensor("XT", [DIN, T * BL], bf16, kind="ExternalInput")
    wit_d = nc.dram_tensor("WIT", [DIN, HID], bf16, kind="ExternalInput")
    wht_d = nc.dram_tensor("WHT", [HID, HID], bf16, kind="ExternalInput")
    wot_d = nc.dram_tensor("WOT", [HID, DOUT], bf16, kind="ExternalInput")
    bh_d = nc.dram_tensor("BH", [128, 1], f32, kind="ExternalInput")    # bias rows 0:128
    bhr_d = nc.dram_tensor("BHR", [1, HID], bf16, kind="ExternalInput")  # bias as row
    bo_d = nc.dram_tensor("BO", [128, 2], f32, kind="ExternalInput")
    yt_d = nc.dram_tensor("YT", [DOUT, T * BL], f32, kind="ExternalOutput")
    hl_d = nc.dram_tensor("HL", [HID, BL], f32, kind="ExternalOutput")

    with tile.TileContext(nc) as tc:
        import contextlib
        ctx = contextlib.ExitStack()
        with ctx:
            if nrep > 1:
                ctx.enter_context(tc.For_i(0, nrep, 1))
            const_p = ctx.enter_context(tc.tile_pool(name="const", bufs=1))
            state_p = ctx.enter_context(tc.tile_pool(name="state", bufs=1))
            xt_p = ctx.enter_context(tc.tile_pool(name="xt", bufs=2))
            hc_p = ctx.enter_context(tc.tile_pool(name="hc", bufs=2))
            yt_p = ctx.enter_context(tc.tile_pool(name="yt", bufs=2))
            pm0_p = ctx.enter_context(tc.tile_pool(name="pm0", bufs=3, space="PSUM"))
            pm123_p = ctx.enter_context(tc.tile_pool(name="pm123", bufs=2, space="PSUM"))
            pwo_p = ctx.enter_context(tc.tile_pool(name="pwo", bufs=2, space="PSUM"))

            # ---- persistent tiles ----
            wit = const_p.tile([128, 2, HID], bf16, tag="wit")
            for k in range(2):
                nc.sync.dma_start(wit[:, k, :], wit_d[128 * k:128 * (k + 1), :])
            wht = const_p.tile([128, 4, HID], bf16, tag="wht")
            for j in range(4):
                nc.sync.dma_start(wht[:, j, :], wht_d[128 * j:128 * (j + 1), :])
            wot = const_p.tile([128, 4, DOUT], bf16, tag="wot")
            for j in range(4):
                nc.sync.dma_start(wot[:, j, :], wot_d[128 * j:128 * (j + 1), :])
            bh = const_p.tile([128, 1], f32, tag="bh")
            nc.sync.dma_start(bh[:], bh_d[:])
            bhr = const_p.tile([1, HID], bf16, tag="bhr")
            nc.sync.dma_start(bhr[:], bhr_d[:])
            bo = const_p.tile([128, 2], f32, tag="bo")
            nc.sync.dma_start(bo[:], bo_d[:])
            ones = const_p.tile([1, BL], bf16, tag="ones")
            nc.gpsimd.memset(ones[:], 1.0)


            mm = nc.tensor.matmul
            act = nc.scalar.activation

            def emit_offchain(t, s, xt_sb, hcol):
                """Matmuls of step t with no dependency on step t-1's tanh.
                hcol(dt) -> (hc_tile, col) for the state column at step t-dt.
                These are emitted BEFORE step t-1's chain matmul so the
                in-order PE sequencer dispatches them while ACT works."""
                a = _active_blocks(t)
                R = NSTATES * a
                M0 = min(R, 128)
                mt = (R + 127) // 128
                xcol = xt_sb[:, :, s * BL:(s + 1) * BL]  # [128, 2, BL]
                # blocks 2..7 (k-tiles 1..3) last written at quad base
                qh, qs = hcol(((t - 1) % 4) + 1) if t > 0 else (None, None)

                pm = None
                if mt > 1:
                    # slow m-tiles: all inputs are >= 4 steps stale
                    pm = pm123_p.tile([128, 3, BL], f32, tag="pm123",
                                      name=f"pm123_{t}")
                    for i in range(1, mt):
                        Mi = min(128, R - 128 * i)
                        o = pm[0:Mi, i - 1, :]
                        gemms = [(bhr[0:1, 128 * i:128 * i + Mi], ones[0:1, :])]
                        for k in range(2):
                            gemms.append(
                                (wit[:, k, 128 * i:128 * i + Mi], xcol[:, k, :]))
                        if t > 0:
                            for j in range(1, 4):
                                if j >= i:
                                    gemms.append(
                                        (wht[:, j, 128 * i:128 * i + Mi],
                                         qh[:, j, qs, :]))
                        for gi, (l_, r_) in enumerate(gemms):
                            mm(o, l_, r_, start=(gi == 0),
                               stop=(gi == len(gemms) - 1))

                p0 = pm0_p.tile([128, BL], f32, tag="pm0", name=f"pm0_{t}")
                o0 = p0[0:M0, :]
                mm(o0, wit[:, 0, 0:M0], xcol[:, 0, :], start=True, stop=False)
                mm(o0, wit[:, 1, 0:M0], xcol[:, 1, :], start=False,
                   stop=(t == 0))
                if t > 0 and (t - 1) % 4 != 0:
                    # quad column is stale (not written at t-1): hoist
                    for j in range(1, 4):
                        mm(o0, wht[:, j, 0:M0], qh[:, j, qs, :],
                           start=False, stop=False)
                if t > 0 and t % 2 == 0:
                    # block1 last written at t-2 (even step)
                    bh_, bs = hcol(2)
                    mm(o0, wht[64:128, 0, 0:M0], bh_[64:128, 0, bs, :],
                       start=False, stop=False)
                if t > 0 and t % 4 == 0 and R < HID:
                    # carry forward the inactive slow blocks into this quad
                    # column (tanh only writes the active prefix)
                    hc4, s4 = hcol(4)
                    hc0, s0 = hcol(0)
                    slot = R // 128
                    if R % 128 == 64:
                        nc.vector.tensor_copy(hc0[64:128, slot, s0, :],
                                              hc4[64:128, slot, s4, :])
                        slot += 1
                    if slot < 4:
                        nc.vector.tensor_copy(hc0[:, slot:4, s0, :],
                                              hc4[:, slot:4, s4, :])
                return p0, pm

            def emit_chain(t, s, hc, p0, pm, hcol):
                """Chain matmul(s) of step t + tanh writing into hc col s."""
                a = _active_blocks(t)
                R = NSTATES * a
                M0 = min(R, 128)
                mt = (R + 127) // 128
                o0 = p0[0:M0, :]
                if t > 0:
                    if (t - 1) % 4 == 0:
                        # quad column was rewritten at step t-1
                        qh, qs = hcol(1)
                        for j in range(1, 4):
                            mm(o0, wht[:, j, 0:M0], qh[:, j, qs, :],
                               start=False, stop=False)
                    ph, ps = hcol(1)
                    if t % 2 == 1:
                        # odd step: block0+block1 both written at t-1 (even)
                        mm(o0, wht[:, 0, 0:M0], ph[:, 0, ps, :],
                           start=False, stop=True)
                    else:
                        mm(o0, wht[0:64, 0, 0:M0], ph[0:64, 0, ps, :],
                           start=False, stop=True)

                if mt > 1:
                    # slow-tile tanh first: its psum is long done, so it runs
                    # in ACT's idle gap while the chain matmul is in flight
                    nfull = (R - 128) // 128  # full m-tiles among 1..3
                    if nfull > 0:
                        act(hc[:, 1:1 + nfull, s, :], pm[:, 0:nfull, :], TANH)
                    if R % 128 != 0:
                        act(hc[0:64, 1 + nfull, s, :], pm[0:64, nfull, :], TANH)
                act(hc[0:M0, 0, s, :], p0[0:M0, :], TANH, bias=bh[0:M0, 0:1])

            def emit_bulk_fill(hc):
                # block1: odd columns <- preceding even column
                nc.vector.tensor_copy(
                    hc[64:128, 0:1, 1:TC:2, :], hc[64:128, 0:1, 0:TC:2, :])
                # k-tiles 1..3: columns s%4==off <- quad base column
                for off in (1, 2, 3):
                    nc.vector.tensor_copy(
                        hc[:, 1:4, off:TC:4, :], hc[:, 1:4, 0:TC:4, :])

            # Wo for one chunk: emits list of (kind, payload) items to be
            # interleaved between recurrence steps.
            def wo_items(hc, yt_sb, cprev):
                items = []
                NG = 4          # n-groups of 512 columns
                for ng in range(NG):
                    scol = ng * 16
                    for m in range(2):
                        state = {}

                        def mk_mm(ng=ng, m=m, scol=scol, state=state):
                            ps = pwo_p.tile([128, 512], f32, tag="pwo",
                                            name=f"pwo_{ng}_{m}")
                            for j in range(4):
                                mm(ps[:, :], wot[:, j, 128 * m:128 * (m + 1)],
                                   hc[:, j, scol:scol + 16, :],
                                   start=(j == 0), stop=(j == 3))
                            state["ps"] = ps

                        def mk_tanh(half, ng=ng, m=m, state=state):
                            ps = state["ps"]
                            c0 = ng * 512 + half * 256
                            act(yt_sb[:, m, c0:c0 + 256],
                                ps[:, half * 256:half * 256 + 256],
                                TANH, bias=bo[:, m:m + 1])

                        items.append(mk_mm)
                        items.append(lambda h=0, f=mk_tanh: f(0))
                        items.append(lambda f=mk_tanh: f(1))
                return items

            xt_tiles = {}

            def load_chunk(c):
                if c in xt_tiles or c >= NCH:
                    return
                xt_sb = xt_p.tile([128, 2, TC * BL], bf16, tag="xt")
                for k in range(2):
                    nc.sync.dma_start(
                        xt_sb[:, k, :],
                        xt_d[128 * k:128 * (k + 1), c * TC * BL:(c + 1) * TC * BL])
                xt_tiles[c] = xt_sb

            prev_hc = None
            prev_c = -1
            pending = None  # (t, s, hc, p0, pm, hcol)
            load_chunk(0)
            for c in range(NCH):
                load_chunk(c + 1)
                hc = hc_p.tile([128, 4, TC, BL], bf16, tag="hc", name=f"hc_{c}")
                items = []
                yt_sb = None
                emitted = 0
                hc_pair = (prev_hc, hc)

                def mk_hcol(s, hc_pair=hc_pair):
                    def hcol(dt):
                        if s - dt >= 0:
                            return hc_pair[1], s - dt
                        return hc_pair[0], TC + s - dt
                    return hcol

                for s in range(TC):
                    t = c * TC + s
                    hcol = mk_hcol(s)
                    p0, pm = emit_offchain(t, s, xt_tiles[c], hcol)
                    if pending is not None:
                        emit_chain(*pending)
                    pending = (t, s, hc, p0, pm, hcol)
                    if s == 1 and prev_hc is not None:
                        # prev chunk's last history column is now written
                        emit_bulk_fill(prev_hc)
                        yt_sb = yt_p.tile([128, 2, TC * BL], f32, tag="yt",
                                          name=f"yt_{c}")
                        items = wo_items(prev_hc, yt_sb, prev_c)
                    if emitted < len(items) and s >= 2 and s % 2 == 1:
                        items[emitted]()
                        emitted += 1
                while emitted < len(items):
                    items[emitted]()
                    emitted += 1
                if yt_sb is not None:
                    for m in range(2):
                        nc.sync.dma_start(
                            yt_d[128 * m:128 * (m + 1),
                                 prev_c * TC * BL:(prev_c + 1) * TC * BL],
                            yt_sb[:, m, :])
                xt_tiles.pop(c, None)
                prev_hc, prev_c = hc, c

            # tail: final chain step + output projection for the last chunk
            emit_chain(*pending)
            emit_bulk_fill(prev_hc)
            yt_sb = yt_p.tile([128, 2, TC * BL], f32, tag="yt", name="yt_tail")
            for it in wo_items(prev_hc, yt_sb, prev_c):
                it()
            for m in range(2):
                nc.sync.dma_start(
                    yt_d[128 * m:128 * (m + 1),
                         prev_c * TC * BL:(prev_c + 1) * TC * BL],
                    yt_sb[:, m, :])

            # H_last: blocks 0/1 from cols 63/62; blocks 2-7 from col 60
            hl_sb = const_p.tile([128, 4, BL], f32, tag="hl")
            nc.vector.tensor_copy(hl_sb[0:64, 0, :], prev_hc[0:64, 0, TC - 1, :])
            nc.vector.tensor_copy(hl_sb[64:128, 0, :],
                                  prev_hc[64:128, 0, TC - 2, :])
            nc.vector.tensor_copy(hl_sb[:, 1:4, :], prev_hc[:, 1:4, TC - 4, :])
            for j in range(4):
                nc.sync.dma_start(hl_d[128 * j:128 * (j + 1), :], hl_sb[:, j, :])

    nc.compile()
    return nc


def _prep_inputs(X, Wi, Wh, Wo):
    bf16 = ml_dtypes.bfloat16
    Wi_w, Wi_b = Wi[:, :-1], Wi[:, -1]
    Wh_w, Wh_b = Wh[:, :-1], Wh[:, -1]
    Wo_w, Wo_b = Wo[:, :-1], Wo[:, -1]
    bias_h = (Wi_b + Wh_b).astype(np.float32)

    WIT = np.ascontiguousarray(Wi_w.T).astype(bf16)          # [DIN, HID]
    WHT = np.ascontiguousarray(Wh_w.T).astype(bf16)          # [HID, HID]
    WOT = np.ascontiguousarray(Wo_w.T).astype(bf16)          # [HID, DOUT]
    BH = bias_h[:128].reshape(128, 1).copy()                 # rows 0:128 (m-tile 0)
    BHR = bias_h.reshape(1, HID).astype(bf16).copy()
    BO = np.ascontiguousarray(Wo_b.reshape(2, 128).T).astype(np.float32)  # [128,2]

    shared = {"WIT": WIT, "WHT": WHT, "WOT": WOT, "BH": BH, "BHR": BHR, "BO": BO}
    in_maps = []
    for c in range(N_CORES):
        Xc = X[:, c * BL:(c + 1) * BL, :]                    # [T, BL, DIN]
        XT = np.ascontiguousarray(Xc.transpose(2, 0, 1).reshape(DIN, T * BL))
        m = dict(shared)
        m["XT"] = XT.astype(bf16)
        in_maps.append(m)
    return in_maps


def kernel(X, Wi, Wh, Wo):
    from concourse.bass_utils import run_bass_kernel_spmd

    X = np.asarray(X, np.float32)
    Wi = np.asarray(Wi, np.float32)
    Wh = np.asarray(Wh, np.float32)
    Wo = np.asarray(Wo, np.float32)

    if "nc" not in _CACHE:
        _CACHE["nc"] = _build_program()
    nc = _CACHE["nc"]

    in_maps = _prep_inputs(X, Wi, Wh, Wo)
    res = run_bass_kernel_spmd(nc, in_maps, core_ids=list(range(N_CORES)))
    _CACHE["last_res"] = res

    Ys = np.empty((T, B, DOUT), np.float32)
    H_last = np.empty((HID, B), np.float32)
    for c in range(N_CORES):
        YT = res.results[c]["YT"]                            # [DOUT, T*BL]
        Ys[:, c * BL:(c + 1) * BL, :] = (
            YT.reshape(DOUT, T, BL).transpose(1, 2, 0))
        H_last[:, c * BL:(c + 1) * BL] = res.results[c]["HL"]
    return Ys, H_last


if __name__ == "__main__":
    rng = np.random.default_rng(0)
    X = rng.standard_normal((T, B, DIN), dtype=np.float32)
    Wi = (rng.standard_normal((HID, DIN + 1), dtype=np.float32) * 0.05)
    Wh = (rng.standard_normal((HID, HID + 1), dtype=np.float32) * 0.05)
    Wo = (rng.standard_normal((DOUT, HID + 1), dtype=np.float32) * 0.05)
    Ys, Hl = kernel(X, Wi, Wh, Wo)
    print("ok", Ys.shape, Hl.shape, float(np.abs(Ys).mean()))


# revision 15
# speedup vs baseline: 1.0862x; 1.0862x over previous
"""Clockwork-RNN Trainium2 kernel (8-core data-parallel over batch).

Per-core layout (B_local = 32 batch columns per core):
  - Host pre-transposes X to d-major ("XT" [DIN, T*BL], bf16) and weights to
    lhsT layouts (WiT/WhT/WoT), so no on-chip transposes are needed.
  - Recurrence runs as a serial chain per timestep: PE matmuls accumulate the
    pre-activation for the *active prefix* of hidden blocks (clockwork
    schedule => active rows are always a prefix), ACT applies tanh and
    overwrites the prefix of the state (binary {0,1} gating == prefix
    overwrite, so no gating arithmetic is needed).
  - tanh writes straight into the per-chunk history buffer hc[*, ktile, s, b];
    the recurrence reads its operands back out of hc at statically-known
    columns (block0 from s-1, block1 from the last even step, slow blocks
    from the last quad step), so there are no per-step state copies at all.
  - Matmuls with no dependency on the previous step's tanh are emitted
    *before* that tanh's chain matmul, so the in-order PE sequencer
    dispatches them during the ACT latency (software pipelining).
  - Slow blocks inactive at a quad step are carried forward by off-chain DVE
    copies; non-quad history columns are bulk-filled at chunk end with
    strided copies (only the output projection reads those).
  - Output projection is computed in flipped orientation (Y [DOUT, T*BL]) so
    Wo stays the stationary operand and the per-partition bias trick works;
    the host transposes Y back. Its tanh is split into halves that hide in
    ACT's idle gaps between chain tanhs.
  - Everything except psum/tanh/Y runs in bf16 (rel err ~3.5e-3 end-to-end).
"""

import numpy as np
import ml_dtypes

T, B, DIN = 1024, 256, 256
NSTATES = 64
NCLOCKS = 8
HID = NCLOCKS * NSTATES  # 512
DOUT = 256
N_CORES = 8
BL = B // N_CORES  # 32
TC = 64            # timesteps per chunk
NCH = T // TC

F32 = None  # set in _build (mybir import deferred)
BF16 = None

_CACHE = {}


def _active_blocks(t: int) -> int:
    if t == 0:
        return NCLOCKS
    return min((t & -t).bit_length() - 1, NCLOCKS - 1) + 1


def _build_program(nrep=1):
    """Build the (SPMD) bass program once; returns (nc, meta)."""
    import concourse.bacc as bacc
    import concourse.tile as tile
    import concourse.mybir as mybir
    from concourse.bass_utils import run_bass_kernel_spmd  # noqa: F401

    f32 = mybir.dt.float32
    bf16 = mybir.dt.bfloat16
    TANH = mybir.ActivationFunctionType.Tanh

    nc = bacc.Bacc("TRN2", target_bir_lowering=False, debug=False)

    xt_d = nc.dram_tensor("XT", [DIN, T * BL], bf16, kind="ExternalInput")
    wit_d = nc.dram_tensor("WIT", [DIN, HID], bf16, kind="ExternalInput")
    wht_d = nc.dram_tensor("WHT", [HID, HID], bf16, kind="ExternalInput")
    wot_d = nc.dram_tensor("WOT", [HID, DOUT], bf16, kind="ExternalInput")
    bh_d = nc.dram_tensor("BH", [128, 1], f32, kind="ExternalInput")    # bias rows 0:128
    bhr_d = nc.dram_tensor("BHR", [1, HID], bf16, kind="ExternalInput")  # bias as row
    bo_d = nc.dram_tensor("BO", [128, 2], f32, kind="ExternalInput")
    yt_d = nc.dram_tensor("YT", [DOUT, T * BL], f32, kind="ExternalOutput")
    hl_d = nc.dram_tensor("HL", [HID, BL], f32, kind="ExternalOutput")

    with tile.TileContext(nc) as tc:
        import contextlib
        ctx = contextlib.ExitStack()
        with ctx:
            if nrep > 1:
                ctx.enter_context(tc.For_i(0, nrep, 1))
            const_p = ctx.enter_context(tc.tile_pool(name="const", bufs=1))
            state_p = ctx.enter_context(tc.tile_pool(name="state", bufs=1))
            xt_p = ctx.enter_context(tc.tile_pool(name="xt", bufs=2))
            hc_p = ctx.enter_context(tc.tile_pool(name="hc", bufs=2))
            yt_p = ctx.enter_context(tc.tile_pool(name="yt", bufs=2))
            pm0_p = ctx.enter_context(tc.tile_pool(name="pm0", bufs=3, space="PSUM"))
            pm123_p = ctx.enter_context(tc.tile_pool(name="pm123", bufs=2, space="PSUM"))
            pwo_p = ctx.enter_context(tc.tile_pool(name="pwo", bufs=2, space="PSUM"))

            # ---- persistent tiles ----
            wit = const_p.tile([128, 2, HID], bf16, tag="wit")
            for k in range(2):
                nc.sync.dma_start(wit[:, k, :], wit_d[128 * k:128 * (k + 1), :])
            wht = const_p.tile([128, 4, HID], bf16, tag="wht")
            for j in range(4):
                nc.sync.dma_start(wht[:, j, :], wht_d[128 * j:128 * (j + 1), :])
            wot = const_p.tile([128, 4, DOUT], bf16, tag="wot")
            for j in range(4):
                nc.sync.dma_start(wot[:, j, :], wot_d[128 * j:128 * (j + 1), :])
            bh = const_p.tile([128, 1], f32, tag="bh")
            nc.sync.dma_start(bh[:], bh_d[:])
            bhr = const_p.tile([1, HID], bf16, tag="bhr")
            nc.sync.dma_start(bhr[:], bhr_d[:])
            bo = const_p.tile([128, 2], f32, tag="bo")
            nc.sync.dma_start(bo[:], bo_d[:])
            ones = const_p.tile([1, BL], bf16, tag="ones")
            nc.gpsimd.memset(ones[:], 1.0)


            mm = nc.tensor.matmul
            act = nc.scalar.activation

            def emit_offchain(t, s, xt_sb, hcol):
                """Matmuls of step t with no dependency on step t-1's tanh.
                hcol(dt) -> (hc_tile, col) for the state column at step t-dt.
                These are emitted BEFORE step t-1's chain matmul so the
                in-order PE sequencer dispatches them while ACT works."""
                a = _active_blocks(t)
                R = NSTATES * a
                M0 = min(R, 128)
                mt = (R + 127) // 128
                xcol = xt_sb[:, :, s * BL:(s + 1) * BL]  # [128, 2, BL]
                # blocks 2..7 (k-tiles 1..3) last written at quad base
                qh, qs = hcol(((t - 1) % 4) + 1) if t > 0 else (None, None)

                pm = None
                if mt > 1:
                    # slow m-tiles: all inputs are >= 4 steps stale
                    pm = pm123_p.tile([128, 3, BL], f32, tag="pm123",
                                      name=f"pm123_{t}")
                    for i in range(1, mt):
                        Mi = min(128, R - 128 * i)
                        o = pm[0:Mi, i - 1, :]
                        gemms = [(bhr[0:1, 128 * i:128 * i + Mi], ones[0:1, :])]
                        for k in range(2):
                            gemms.append(
                                (wit[:, k, 128 * i:128 * i + Mi], xcol[:, k, :]))
                        if t > 0:
                            for j in range(1, 4):
                                if j >= i:
                                    gemms.append(
                                        (wht[:, j, 128 * i:128 * i + Mi],
                                         qh[:, j, qs, :]))
                        for gi, (l_, r_) in enumerate(gemms):
                            mm(o, l_, r_, start=(gi == 0),
                               stop=(gi == len(gemms) - 1))

                p0 = pm0_p.tile([128, BL], f32, tag="pm0", name=f"pm0_{t}")
                o0 = p0[0:M0, :]
                mm(o0, wit[:, 0, 0:M0], xcol[:, 0, :], start=True, stop=False)
                mm(o0, wit[:, 1, 0:M0], xcol[:, 1, :], start=False,
                   stop=(t == 0))
                if t > 0 and (t - 1) % 4 != 0:
                    # quad column is stale (not written at t-1): hoist
                    for j in range(1, 4):
                        mm(o0, wht[:, j, 0:M0], qh[:, j, qs, :],
                           start=False, stop=False)
                if t > 0 and t % 2 == 0:
                    # block1 last written at t-2 (even step)
                    bh_, bs = hcol(2)
                    mm(o0, wht[64:128, 0, 0:M0], bh_[64:128, 0, bs, :],
                       start=False, stop=False)
                if t > 0 and t % 4 == 0 and R < HID:
                    # carry forward the inactive slow blocks into this quad
                    # column (tanh only writes the active prefix)
                    hc4, s4 = hcol(4)
                    hc0, s0 = hcol(0)
                    slot = R // 128
                    if R % 128 == 64:
                        nc.vector.tensor_copy(hc0[64:128, slot, s0, :],
                                              hc4[64:128, slot, s4, :])
                        slot += 1
                    if slot < 4:
                        nc.vector.tensor_copy(hc0[:, slot:4, s0, :],
                                              hc4[:, slot:4, s4, :])
                return p0, pm

            def emit_chain(t, s, hc, p0, pm, hcol):
                """Chain matmul(s) of step t + tanh writing into hc col s."""
                a = _active_blocks(t)
                R = NSTATES * a
                M0 = min(R, 128)
                mt = (R + 127) // 128
                o0 = p0[0:M0, :]
                if t > 0:
                    if (t - 1) % 4 == 0:
                        # quad column was rewritten at step t-1
                        qh, qs = hcol(1)
                        for j in range(1, 4):
                            mm(o0, wht[:, j, 0:M0], qh[:, j, qs, :],
                               start=False, stop=False)
                    ph, ps = hcol(1)
                    if t % 2 == 1:
                        # odd step: block0+block1 both written at t-1 (even)
                        mm(o0, wht[:, 0, 0:M0], ph[:, 0, ps, :],
                           start=False, stop=True)
                    else:
                        mm(o0, wht[0:64, 0, 0:M0], ph[0:64, 0, ps, :],
                           start=False, stop=True)

                if mt > 1:
                    # slow-tile tanh first: its psum is long done, so it runs
                    # in ACT's idle gap while the chain matmul is in flight
                    nfull = (R - 128) // 128  # full m-tiles among 1..3
                    if nfull > 0:
                        act(hc[:, 1:1 + nfull, s, :], pm[:, 0:nfull, :], TANH)
                    if R % 128 != 0:
                        act(hc[0:64, 1 + nfull, s, :], pm[0:64, nfull, :], TANH)
                act(hc[0:M0, 0, s, :], p0[0:M0, :], TANH, bias=bh[0:M0, 0:1])

            def emit_bulk_fill(hc):
                # block1: odd columns <- preceding even column
                nc.vector.tensor_copy(
                    hc[64:128, 0:1, 1:TC:2, :], hc[64:128, 0:1, 0:TC:2, :])
                # k-tiles 1..3: columns s%4==off <- quad base column
                for off in (1, 2, 3):
                    nc.vector.tensor_copy(
                        hc[:, 1:4, off:TC:4, :], hc[:, 1:4, 0:TC:4, :])

            # Wo for one chunk: emits list of (kind, payload) items to be
            # interleaved between recurrence steps.
            def wo_items(hc, yt_sb, cprev):
                # one item per matmul / half-tanh so each fits in the PE/ACT
                # idle window of a single recurrence step
                items = []
                NG = 4          # n-groups of 512 columns
                for ng in range(NG):
                    scol = ng * 16
                    for m in range(2):
                        state = {}

                        def mk_mm(j, ng=ng, m=m, scol=scol, state=state):
                            if j == 0:
                                state["ps"] = pwo_p.tile(
                                    [128, 512], f32, tag="pwo",
                                    name=f"pwo_{ng}_{m}")
                            mm(state["ps"][:, :],
                               wot[:, j, 128 * m:128 * (m + 1)],
                               hc[:, j, scol:scol + 16, :],
                               start=(j == 0), stop=(j == 3))

                        def mk_tanh(half, ng=ng, m=m, state=state):
                            ps = state["ps"]
                            c0 = ng * 512 + half * 256
                            act(yt_sb[:, m, c0:c0 + 256],
                                ps[:, half * 256:half * 256 + 256],
                                TANH, bias=bo[:, m:m + 1])

                        for j in range(4):
                            items.append(lambda j=j, f=mk_mm: f(j))
                        items.append(lambda f=mk_tanh: f(0))
                        items.append(lambda f=mk_tanh: f(1))
                return items

            xt_tiles = {}

            def load_chunk(c):
                if c in xt_tiles or c >= NCH:
                    return
                xt_sb = xt_p.tile([128, 2, TC * BL], bf16, tag="xt")
                for k in range(2):
                    nc.sync.dma_start(
                        xt_sb[:, k, :],
                        xt_d[128 * k:128 * (k + 1), c * TC * BL:(c + 1) * TC * BL])
                xt_tiles[c] = xt_sb

            prev_hc = None
            prev_c = -1
            pending = None  # (t, s, hc, p0, pm, hcol)
            load_chunk(0)
            for c in range(NCH):
                load_chunk(c + 1)
                hc = hc_p.tile([128, 4, TC, BL], bf16, tag="hc", name=f"hc_{c}")
                items = []
                yt_sb = None
                emitted = 0
                hc_pair = (prev_hc, hc)

                def mk_hcol(s, hc_pair=hc_pair):
                    def hcol(dt):
                        if s - dt >= 0:
                            return hc_pair[1], s - dt
                        return hc_pair[0], TC + s - dt
                    return hcol

                for s in range(TC):
                    t = c * TC + s
                    hcol = mk_hcol(s)
                    p0, pm = emit_offchain(t, s, xt_tiles[c], hcol)
                    if emitted < len(items) and s >= 2:
                        # output-projection work goes BEFORE the chain matmul
                        # in engine order so it streams during the tanh window
                        items[emitted]()
                        emitted += 1
                    if pending is not None:
                        emit_chain(*pending)
                    pending = (t, s, hc, p0, pm, hcol)
                    if s == 1 and prev_hc is not None:
                        # prev chunk's last history column is now written
                        emit_bulk_fill(prev_hc)
                        yt_sb = yt_p.tile([128, 2, TC * BL], f32, tag="yt",
                                          name=f"yt_{c}")
                        items = wo_items(prev_hc, yt_sb, prev_c)
                while emitted < len(items):
                    items[emitted]()
                    emitted += 1
                if yt_sb is not None:
                    for m in range(2):
                        nc.sync.dma_start(
                            yt_d[128 * m:128 * (m + 1),
                                 prev_c * TC * BL:(prev_c + 1) * TC * BL],
                            yt_sb[:, m, :])
                xt_tiles.pop(c, None)
                prev_hc, prev_c = hc, c

            # tail: final chain step + output projection for the last chunk
            emit_chain(*pending)
            emit_bulk_fill(prev_hc)
            yt_sb = yt_p.tile([128, 2, TC * BL], f32, tag="yt", name="yt_tail")
            for it in wo_items(prev_hc, yt_sb, prev_c):
                it()
            for m in range(2):
                nc.sync.dma_start(
                    yt_d[128 * m:128 * (m + 1),
                         prev_c * TC * BL:(prev_c + 1) * TC * BL],
                    yt_sb[:, m, :])

            # H_last: blocks 0/1 from cols 63/62; blocks 2-7 from col 60
            hl_sb = const_p.tile([128, 4, BL], f32, tag="hl")
            nc.vector.tensor_copy(hl_sb[0:64, 0, :], prev_hc[0:64, 0, TC - 1, :])
            nc.vector.tensor_copy(hl_sb[64:128, 0, :],
                                  prev_hc[64:128, 0, TC - 2, :])
            nc.vector.tensor_copy(hl_sb[:, 1:4, :], prev_hc[:, 1:4, TC - 4, :])
            for j in range(4):
                nc.sync.dma_start(hl_d[128 * j:128 * (j + 1), :], hl_sb[:, j, :])

    nc.compile()
    return nc


def _prep_inputs(X, Wi, Wh, Wo):
    bf16 = ml_dtypes.bfloat16
    Wi_w, Wi_b = Wi[:, :-1], Wi[:, -1]
    Wh_w, Wh_b = Wh[:, :-1], Wh[:, -1]
    Wo_w, Wo_b = Wo[:, :-1], Wo[:, -1]
    bias_h = (Wi_b + Wh_b).astype(np.float32)

    WIT = np.ascontiguousarray(Wi_w.T).astype(bf16)          # [DIN, HID]
    WHT = np.ascontiguousarray(Wh_w.T).astype(bf16)          # [HID, HID]
    WOT = np.ascontiguousarray(Wo_w.T).astype(bf16)          # [HID, DOUT]
    BH = bias_h[:128].reshape(128, 1).copy()                 # rows 0:128 (m-tile 0)
    BHR = bias_h.reshape(1, HID).astype(bf16).copy()
    BO = np.ascontiguousarray(Wo_b.reshape(2, 128).T).astype(np.float32)  # [128,2]

    shared = {"WIT": WIT, "WHT": WHT, "WOT": WOT, "BH": BH, "BHR": BHR, "BO": BO}
    in_maps = []
    for c in range(N_CORES):
        Xc = X[:, c * BL:(c + 1) * BL, :]                    # [T, BL, DIN]
        XT = np.ascontiguousarray(Xc.transpose(2, 0, 1).reshape(DIN, T * BL))
        m = dict(shared)
        m["XT"] = XT.astype(bf16)
        in_maps.append(m)
    return in_maps


def kernel(X, Wi, Wh, Wo):
    from concourse.bass_utils import run_bass_kernel_spmd

    X = np.asarray(X, np.float32)
    Wi = np.asarray(Wi, np.float32)
    Wh = np.asarray(Wh, np.float32)
    Wo = np.asarray(Wo, np.float32)

    if "nc" not in _CACHE:
        _CACHE["nc"] = _build_program()
    nc = _CACHE["nc"]

    in_maps = _prep_inputs(X, Wi, Wh, Wo)
    res = run_bass_kernel_spmd(nc, in_maps, core_ids=list(range(N_CORES)))
    _CACHE["last_res"] = res

    Ys = np.empty((T, B, DOUT), np.float32)
    H_last = np.empty((HID, B), np.float32)
    for c in range(N_CORES):
        YT = res.results[c]["YT"]                            # [DOUT, T*BL]
        Ys[:, c * BL:(c + 1) * BL, :] = (
            YT.reshape(DOUT, T, BL).transpose(1, 2, 0))
        H_last[:, c * BL:(c + 1) * BL] = res.results[c]["HL"]
    return Ys, H_last


if __name__ == "__main__":
    rng = np.random.default_rng(0)
    X = rng.standard_normal((T, B, DIN), dtype=np.float32)
    Wi = (rng.standard_normal((HID, DIN + 1), dtype=np.float32) * 0.05)
    Wh = (rng.standard_normal((HID, HID + 1), dtype=np.float32) * 0.05)
    Wo = (rng.standard_normal((DOUT, HID + 1), dtype=np.float32) * 0.05)
    Ys, Hl = kernel(X, Wi, Wh, Wo)
    print("ok", Ys.shape, Hl.shape, float(np.abs(Ys).mean()))
